# revision 6
# baseline (speedup 1.0000x reference)
"""Grouped GEMM (MoE routing) kernel for 8 Trainium2 NeuronCores.

Computation: for expert e, rows seg_indptr[e]:seg_indptr[e+1] of a[M,K] are
multiplied by b[e]^T (b is [E,N,K]), then scaled per-token (scale_a) and
per-expert (scale_b).

Strategy: 2D grid of 4 N-groups x 2 K-halves across the 8 cores. Core c
handles N columns [g*1408, (g+1)*1408) (g = c//2) for K rows
[h*1024, (h+1)*1024) (h = c%2), over ALL M token rows; the host sums the two
K-half partials per N-group. 1408 = 11*128 exactly, so every stationary
weight chunk is full 128 wide (no PE column waste), and K=1024 = 8*128.

The per-expert segment structure (from seg_indptr, known on host at call
time) is baked into a single SPMD program shared by all 8 cores; per-core
differences are input *values* only. Scales are folded into `a` on the host
(row scaling commutes with the GEMM). Matmuls run as float32r (full-rate
fp32 path; needs even moving size, >=256 to avoid the 1/4-rate mode).

All device tensors are host-packed into DMA-friendly tiled layouts so each
SBUF-partition line is one large contiguous descriptor:
  a [NCH, 128, 8, 512]  a[ci, p, kc, m] = a_scaled[m0_ci + m, h*1024 + kc*128 + p]
  w [E, 128, 8, 1408]   w[e, p, kc, n]  = b[e, g*1408 + n, h*1024 + kc*128 + p]
  o [NCH, 128, 11, 512] o[ci, p, ch, m] = out_partial[m0_ci + m, g*1408 + ch*128 + p]
Loads issue on the sync engine's HWDGE ring, stores on the scalar engine's,
so descriptor generation is split across both rings.
"""

import sys

import numpy as np

_TRN = "/opt/trn_rl_repo"
if _TRN not in sys.path:
    sys.path.insert(0, _TRN)

M, K, N, E = 16384, 2048, 5632, 8
NCORES = 8
NGROUPS = 4  # N split
NSLICE = N // NGROUPS  # 1408 = 11 * 128
NCH_N = NSLICE // 128  # 11
KHALF = K // 2  # 1024
KC = KHALF // 128  # 8
P = 128
MCHUNK = 512

_cache: dict = {}


def _chunks_of(segs):
    """[(m0, mjw, mjw_mm)] for all experts' m-chunks + per-expert count.

    Chunk sizes are balanced per expert (all <= 512, near-equal) so there is
    no padded-to-512 tail. mjw_mm is the fp32r moving size: even, and the
    cheaper of "pad to 256 (full rate)" vs "stay small (1/4 rate)".
    """
    chunks = []
    counts = []
    for m_start, m_len in segs:
        if m_len == 0:
            counts.append(0)
            continue
        cnt = -(-m_len // MCHUNK)
        s = 2 * (-(-m_len // (2 * cnt)))  # even, balanced
        sizes = [s] * (cnt - 1) + [m_len - s * (cnt - 1)]
        m0 = m_start
        for mjw in sizes:
            even = mjw + (mjw & 1)
            mjw_mm = even if (even >= 256 or 4 * even < 256) else 256
            chunks.append((m0, mjw, mjw_mm))
            m0 += mjw
        counts.append(cnt)
    return chunks, counts


def _build_program(segs):
    from concourse import bacc
    import concourse.mybir as mybir
    import concourse.tile as tile

    f32 = mybir.dt.float32
    f32r = mybir.dt.float32r

    chunks, counts = _chunks_of(segs)
    nch = len(chunks)

    nc = bacc.Bacc(name="grouped_gemm")
    a_p = nc.declare_dram_parameter("a", [nch, P, KC, MCHUNK], f32r, isOutput=False)
    w_p = nc.declare_dram_parameter("w", [E, P, KC, NSLICE], f32r, isOutput=False)
    o_p = nc.declare_dram_parameter("o", [nch, P, NCH_N, MCHUNK], f32, isOutput=True)

    with (
        tile.TileContext(nc) as tc,
        tc.tile_pool(name="wp", bufs=2) as wp,
        tc.tile_pool(name="apool", bufs=3) as apool,
        tc.tile_pool(name="spool", bufs=2) as spool,
        tc.tile_pool(name="pspool", bufs=8, space="PSUM") as pspool,
    ):
        ci = 0
        first = True
        for e in range(E):
            if counts[e] == 0:
                continue
            w_t = wp.tile([P, KC, NSLICE], f32r, tag="w")
            if first:
                # Split the first loads by k-chunk so the first matmul (which
                # only needs kc=0) starts after ~1/8th of the load.
                for kc in range(KC):
                    nc.sync.dma_start(w_t[:, kc, :], w_p[e, :, kc, :])
            else:
                nc.sync.dma_start(w_t[:], w_p[e])
            for _ in range(counts[e]):
                _, mjw, mjw_mm = chunks[ci]
                a_t = apool.tile([P, KC, MCHUNK], f32r, tag="a")
                # Always load the full (zero-padded) 512 so each SBUF
                # partition line is one contiguous 16KB descriptor.
                if first:
                    for kc in range(KC):
                        nc.sync.dma_start(a_t[:, kc, :], a_p[ci, :, kc, :])
                    first = False
                else:
                    nc.sync.dma_start(a_t[:], a_p[ci])
                st = spool.tile([P, NCH_N, MCHUNK], f32, tag="st")
                for ch in range(NCH_N):
                    ps = pspool.tile([P, MCHUNK], f32, tag="ps")
                    for kc in range(KC):
                        nc.tensor.matmul(
                            ps[:, :mjw_mm],
                            w_t[:, kc, ch * P : (ch + 1) * P],
                            a_t[:, kc, :mjw_mm],
                            start=(kc == 0),
                            stop=(kc == KC - 1),
                        )
                    nc.vector.tensor_copy(st[:, ch, :mjw], ps[:, :mjw])
                nc.scalar.dma_start(o_p[ci, :, :, :mjw], st[:, :, :mjw])
                ci += 1

    nc.finalize()
    return nc


def _get_program(segs):
    nc = _cache.get(segs)
    if nc is None:
        nc = _build_program(segs)
        _cache[segs] = nc
    return nc


def kernel(a, b, scale_a, scale_b, seg_indptr, batch_size, _want_trace=False):
    from concourse.bass_utils import run_bass_kernel_spmd

    a = np.asarray(a, dtype=np.float32)
    b = np.asarray(b, dtype=np.float32)
    scale_a = np.asarray(scale_a, dtype=np.float32).reshape(M, 1)
    scale_b = np.asarray(scale_b, dtype=np.float32).reshape(E, 1)
    seg = np.asarray(seg_indptr).astype(np.int64)

    segs = []
    row_scale = np.empty((M, 1), dtype=np.float32)
    for e in range(E):
        s, t = int(seg[e]), int(seg[e + 1])
        s, t = max(0, min(s, M)), max(0, min(t, M))
        segs.append((s, max(0, t - s)))
        if t > s:
            row_scale[s:t] = scale_b[e, 0]
    segs = tuple(segs)
    row_scale *= scale_a

    chunks, _counts = _chunks_of(segs)
    nch = len(chunks)
    nc = _get_program(segs)

    a_scaled = a * row_scale  # [M, K]
    # Pack a chunks per K-half: a_pk[h][ci, p, kc, m]
    a_pk = [np.zeros((nch, P, KC, MCHUNK), dtype=np.float32) for _ in range(2)]
    for ci, (m0, mjw, _mm) in enumerate(chunks):
        blk = a_scaled[m0 : m0 + mjw]  # [mjw, K]
        # [mjw, 2, 8, 128] -> (h, p, kc, m)
        blk4 = blk.reshape(mjw, 2, KC, P).transpose(1, 3, 2, 0)
        a_pk[0][ci, :, :, :mjw] = blk4[0]
        a_pk[1][ci, :, :, :mjw] = blk4[1]

    # Pack weights per core: w[e, p, kc, n] = b[e, g*1408+n, h*1024+kc*128+p]
    in_maps = []
    for c in range(NCORES):
        g, h = c // 2, c % 2
        bw = b[:, g * NSLICE : (g + 1) * NSLICE, h * KHALF : (h + 1) * KHALF]
        # [E, n, kc, p] -> [E, p, kc, n]
        w_c = np.ascontiguousarray(
            bw.reshape(E, NSLICE, KC, P).transpose(0, 3, 2, 1)
        )
        in_maps.append({"a": a_pk[h], "w": w_c})

    res = run_bass_kernel_spmd(
        nc, in_maps, list(range(NCORES)), trace=_want_trace
    )

    out = np.empty((M, N), dtype=np.float32)
    for g in range(NGROUPS):
        o_sum = res.results[2 * g]["o"] + res.results[2 * g + 1]["o"]
        for ci, (m0, mjw, _mm) in enumerate(chunks):
            # [p, ch, m] -> [m, ch, p] -> [mjw, 1408]
            out[m0 : m0 + mjw, g * NSLICE : (g + 1) * NSLICE] = (
                o_sum[ci, :, :, :mjw].transpose(2, 1, 0).reshape(mjw, NSLICE)
            )
    if _want_trace:
        return out, res
    return out
